# revision 2
# baseline (speedup 1.0000x reference)
"""Trainium2 Bass kernel for nn_CMIConnector: visual->ds projection, linear SSM
scan along Lv with time-invariant per-(batch,channel) gates, then out-projection
to d_model. Data-parallel over batch across 8 NeuronCores.

Reference math (per batch row b):
    tc     = mean_Lt(text_embeds[b])                    # [Dt]
    delta  = sigmoid(tc @ Wd.T + bd)                    # [ds]
    B_vec  = tc @ WB.T + bB                             # [ds]
    C_vec  = tc @ WC.T + bC                             # [ds]
    x_t    = visual[b, t] @ Wx.T + bx                   # [ds]
    h_t    = (1-delta) * h_{t-1} + delta*B_vec*x_t      # linear scan over Lv
    out_t  = (C_vec * h_t) @ Wo.T + bo                  # [dm]

Device layout: channels on partitions, time on the free dim, so the SSM maps to
hardware tensor_tensor_scan instructions (chunked along Lv so the out-projection
and its output DMA start streaming almost immediately).

The kernel is HBM-bandwidth bound (per core: visual in + d_model out dominate),
so both sides of the big I/O run in fp16: the host pre-casts visual_feats to
fp16 (halving the input read) and the device stores the output in fp16 which
the host upcasts after gather (halving the 128 MiB/core output write). The
rel-err budget (2e-2) dwarfs fp16 rounding (~1e-3 here). The scan and the
out-projection stay in float32r (full-rate single-pass PE fp32), so only the
visual matmul operands and the final store are 16-bit.
"""

import os
import sys

import numpy as np

for _p in ("/opt/trn_rl_repo",):
    if _p not in sys.path and os.path.isdir(_p):
        sys.path.insert(0, _p)

import concourse.bass as bass  # noqa: E402
import concourse.tile as tile  # noqa: E402
from concourse import bacc, mybir  # noqa: E402
from concourse.bass_utils import run_bass_kernel_spmd  # noqa: E402

F32 = mybir.dt.float32
F32R = mybir.dt.float32r
FP16 = mybir.dt.float16

# Problem shapes (hardcoded per the contract).
B, Lv, Dv = 16, 4096, 1024
Lt, Dt = 128, 4096
DS, DM = 64, 4096
NCORES = 8
BPC = B // NCORES  # batches per core

MM_DTYPE = F32R

NJ = Dt // 128  # gate contraction chunks
ND = Dv // 128  # x-proj contraction chunks
NT = Lv // 512  # time chunks (x-proj / scan granularity)
NH = 2  # visual halves (DMA granularity: one 4 MiB load per half)
HLEN = Lv // NH


def _build_program(mm_dtype=MM_DTYPE):
    nc = bacc.Bacc()
    AF = mybir.ActivationFunctionType
    OP = mybir.AluOpType
    mm = mm_dtype

    # All weight/text tensors are host-packed into their on-chip layouts so
    # every load is one large DMA with contiguous per-partition rows.
    vis16 = nc.dram_tensor("vis16", [BPC, NH, 128, ND * HLEN], FP16, kind="ExternalInput")
    text16 = nc.dram_tensor("text16", [BPC, 128, NJ, Lt], FP16, kind="ExternalInput")
    wxt16 = nc.dram_tensor("wxt16", [128, ND, DS], FP16, kind="ExternalInput")
    wg16 = nc.dram_tensor("wg16", [128, NJ, 3, DS], FP16, kind="ExternalInput")
    wob = nc.dram_tensor("wob", [DS + 1, DM], mm, kind="ExternalInput")
    bd_c = nc.dram_tensor("bd_c", [DS, 1], F32, kind="ExternalInput")
    nbd_c = nc.dram_tensor("nbd_c", [DS, 1], F32, kind="ExternalInput")
    bb_c = nc.dram_tensor("bb_c", [DS, 1], F32, kind="ExternalInput")
    bc_c = nc.dram_tensor("bc_c", [DS, 1], F32, kind="ExternalInput")
    bx_c = nc.dram_tensor("bx_c", [DS, 1], F32, kind="ExternalInput")
    ones_row = nc.dram_tensor("ones_row", [1, Lv], mm, kind="ExternalInput")
    out16 = nc.dram_tensor("out16", [BPC, Lv, DM], FP16, kind="ExternalOutput")

    with tile.TileContext(nc) as tc:
        with (
            tc.tile_pool(name="persist", bufs=1) as persist,
            tc.tile_pool(name="tstream", bufs=2) as tstream,
        ):
            wxt_sb = persist.tile([128, ND, DS], FP16)
            nc.sync.dma_start(out=wxt_sb[:], in_=wxt16[:])
            wg_sb = persist.tile([128, NJ, 3, DS], FP16)
            nc.sync.dma_start(out=wg_sb[:], in_=wg16[:])

            bd_sb = persist.tile([DS, 1], F32)
            nc.sync.dma_start(out=bd_sb[:], in_=bd_c[:])
            nbd_sb = persist.tile([DS, 1], F32)
            nc.sync.dma_start(out=nbd_sb[:], in_=nbd_c[:])
            bb_sb = persist.tile([DS, 1], F32)
            nc.sync.dma_start(out=bb_sb[:], in_=bb_c[:])
            bc_sb = persist.tile([DS, 1], F32)
            nc.sync.dma_start(out=bc_sb[:], in_=bc_c[:])
            bx_sb = persist.tile([DS, 1], F32)
            nc.sync.dma_start(out=bx_sb[:], in_=bx_c[:])

            # ---- Phase 0: fused text-mean gate projections (fp16 PE) ----
            zd_sb = persist.tile([DS, BPC], F32)
            zb_sb = persist.tile([DS, BPC], F32)
            zc_sb = persist.tile([DS, BPC], F32)
            with tc.tile_pool(name="psum0", bufs=2, space="PSUM") as psum0:
                for b in range(BPC):
                    tt = tstream.tile([128, NJ, Lt], FP16, tag="t16")
                    nc.sync.dma_start(out=tt[:], in_=text16[b])
                    zd_ps = psum0.tile([DS, Lt], F32, tag="zd")
                    zb_ps = psum0.tile([DS, Lt], F32, tag="zb")
                    zc_ps = psum0.tile([DS, Lt], F32, tag="zc")
                    for j in range(NJ):
                        for g, ps in enumerate((zd_ps, zb_ps, zc_ps)):
                            nc.tensor.matmul(
                                ps[:],
                                wg_sb[:, j, g, :],
                                tt[:, j, :],
                                start=(j == 0),
                                stop=(j == NJ - 1),
                            )
                    # mean over Lt (1/Lt folded into wg16 on host)
                    nc.vector.reduce_sum(
                        zd_sb[:, b : b + 1], zd_ps[:], axis=mybir.AxisListType.X
                    )
                    nc.vector.reduce_sum(
                        zb_sb[:, b : b + 1], zb_ps[:], axis=mybir.AxisListType.X
                    )
                    nc.vector.reduce_sum(
                        zc_sb[:, b : b + 1], zc_ps[:], axis=mybir.AxisListType.X
                    )

            delta_sb = persist.tile([DS, BPC], F32)
            nc.scalar.activation(
                delta_sb[:], zd_sb[:], AF.Sigmoid, bias=bd_sb[:, 0:1], scale=1.0
            )
            a_sb = persist.tile([DS, BPC], F32)
            nc.scalar.activation(
                a_sb[:], zd_sb[:], AF.Sigmoid, bias=nbd_sb[:, 0:1], scale=-1.0
            )
            bv_sb = persist.tile([DS, BPC], F32)
            nc.vector.tensor_scalar_add(bv_sb[:], zb_sb[:], bb_sb[:, 0:1])
            cv_sb = persist.tile([DS, BPC], F32)
            nc.vector.tensor_scalar_add(cv_sb[:], zc_sb[:], bc_sb[:, 0:1])
            db_sb = persist.tile([DS, BPC], F32)
            nc.vector.tensor_mul(db_sb[:], delta_sb[:], bv_sb[:])
            # Fold the output gate C into the scan input: scanning
            # u'_t = C*delta*B*x_t yields y_t = C*h_t directly.
            cdb_sb = persist.tile([DS, BPC], F32)
            nc.vector.tensor_mul(cdb_sb[:], db_sb[:], cv_sb[:])
            cdbx_sb = persist.tile([DS, BPC], F32)
            nc.vector.tensor_scalar_mul(cdbx_sb[:], cdb_sb[:], bx_sb[:, 0:1])

            # Loaded here (not at the top) so the small gate/x-proj loads win
            # the head of the sync DMA ring and the pipeline starts sooner.
            wo_sb = persist.tile([DS + 1, DM], mm)
            nc.sync.dma_start(out=wo_sb[:], in_=wob[:])

            # ---- Phases 1+2: x-proj + chunked scan + out-proj, per batch ----
            evac_i = [0]

            with (
                tc.tile_pool(name="psx", bufs=2, space="PSUM") as psx,
                tc.tile_pool(name="pso", bufs=3, space="PSUM") as pso,
                tc.tile_pool(name="visb", bufs=2) as visb,
                tc.tile_pool(name="ubp", bufs=2) as ubp,
                tc.tile_pool(name="abp", bufs=2) as abp,
                tc.tile_pool(name="outp", bufs=3) as outp,
            ):
                for b in range(BPC):
                    u_t = ubp.tile([DS, Lv], F32, tag="u")
                    y_r = ubp.tile([DS + 1, Lv], mm, tag="y")
                    nc.sync.dma_start(out=y_r[DS : DS + 1, :], in_=ones_row[:])
                    # per-chunk broadcast of the decay gate a=(1-delta): the
                    # scan consumes the same [DS, 512] columns every chunk.
                    a_bc = abp.tile([DS, 512], F32, tag="a")
                    nc.gpsimd.memset(a_bc[:], 1.0)
                    nc.vector.tensor_scalar_mul(a_bc[:], a_bc[:], a_sb[:, b : b + 1])

                    vis_tiles = {}
                    for h in range(NH):
                        vt = visb.tile([128, ND * HLEN], FP16, tag="v")
                        nc.sync.dma_start(out=vt[:], in_=vis16[b, h])
                        vis_tiles[h] = vt

                    def xproj_scan(t):
                        sl = slice(t * 512, (t + 1) * 512)
                        h, i = divmod(t, NT // NH)
                        vt = vis_tiles[h]
                        xp = psx.tile([DS, 512], F32, tag="x")
                        for d in range(ND):
                            nc.tensor.matmul(
                                xp[:],
                                wxt_sb[:, d, :],
                                vt[:, d * HLEN + i * 512 : d * HLEN + (i + 1) * 512],
                                start=(d == 0),
                                stop=(d == ND - 1),
                            )
                        # u = (C*deltaB) * x_raw + (C*deltaB)*bx
                        nc.scalar.activation(
                            u_t[:, sl],
                            xp[:],
                            AF.Identity,
                            bias=cdbx_sb[:, b : b + 1],
                            scale=cdb_sb[:, b : b + 1],
                        )
                        # chunked scan; chain via the previous chunk's last col
                        nc.vector.tensor_tensor_scan(
                            y_r[0:DS, sl],
                            a_bc[:],
                            u_t[:, sl],
                            0.0 if t == 0 else y_r[0:DS, t * 512 - 1 : t * 512],
                            OP.mult,
                            OP.add,
                        )

                    def outproj(t):
                        for tt_i in range(t * 4, t * 4 + 4):
                            ot = outp.tile([128, DM], FP16, tag="o")
                            lhs = y_r[:, tt_i * 128 : (tt_i + 1) * 128]
                            for nn in range(DM // 1024):
                                op_ = pso.tile([128, 1024], F32, tag="op")
                                for hh in range(2):
                                    nc.tensor.matmul(
                                        op_[:, hh * 512 : (hh + 1) * 512],
                                        lhs,
                                        wo_sb[
                                            :,
                                            nn * 1024 + hh * 512 : nn * 1024
                                            + (hh + 1) * 512,
                                        ],
                                        start=True,
                                        stop=True,
                                    )
                                dst = ot[:, nn * 1024 : (nn + 1) * 1024]
                                # PSUM evacuation (with the f32->fp16 cast) is
                                # split 2/5 scalar : 3/5 vector to match the
                                # engines' elementwise rates (153 vs 245 G/s).
                                if evac_i[0] % 5 in (0, 3):
                                    nc.scalar.activation(dst, op_[:], AF.Copy)
                                else:
                                    nc.vector.tensor_copy(dst, op_[:])
                                evac_i[0] += 1
                            nc.scalar.dma_start(
                                out=out16[b, tt_i * 128 : (tt_i + 1) * 128, :],
                                in_=ot[:],
                            )

                    # Software pipeline: x-proj/scan run one chunk ahead of the
                    # out-projection, so each chunk's scan result is ready the
                    # moment the PE finishes the previous chunk's matmuls and
                    # the output-store stream never stalls at chunk boundaries.
                    xproj_scan(0)
                    for t in range(NT):
                        if t + 1 < NT:
                            xproj_scan(t + 1)
                        outproj(t)
    return nc


def _prep_host_inputs(
    visual_feats, text_embeds, Wx, bx, Wd, bd, WB, bB, WC, bC, Wo, bo
):
    f = lambda a: np.asarray(a, dtype=np.float32)
    # [B, Lv, Dv] -> [B, NH, 128p, ND*HLEN] fp16 with element
    # (b, h, p, d*HLEN+t) = visual[b, h*HLEN+t, d*128+p]
    vis16 = np.ascontiguousarray(
        f(visual_feats)
        .transpose(0, 2, 1)
        .reshape(B, ND, 128, NH, HLEN)
        .transpose(0, 3, 2, 1, 4)
        .reshape(B, NH, 128, ND * HLEN)
        .astype(np.float16)
    )
    # [B, Lt, Dt] -> [B, 128p, NJ, Lt] with Dt index = j*128 + p
    text16 = np.ascontiguousarray(
        f(text_embeds)
        .transpose(0, 2, 1)
        .reshape(B, NJ, 128, Lt)
        .transpose(0, 2, 1, 3)
        .astype(np.float16)
    )
    # Wx.T [Dv, ds] -> [128p, ND, ds] with Dv index = c*128 + p
    wxt16 = np.ascontiguousarray(
        f(Wx).T.reshape(ND, 128, DS).transpose(1, 0, 2).astype(np.float16)
    )
    # Gate weights transposed, pre-scaled by 1/Lt (the text mean), fp16,
    # packed [Dt, 3, ds] -> [128p, NJ, 3, ds] with Dt index = j*128 + p.
    wg16 = np.ascontiguousarray(
        (np.stack([f(Wd).T, f(WB).T, f(WC).T], axis=1) / np.float32(Lt))
        .reshape(NJ, 128, 3, DS)
        .transpose(1, 0, 2, 3)
        .astype(np.float16)
    )
    wob = np.ascontiguousarray(
        np.concatenate([f(Wo).T, f(bo)[None, :]], axis=0)
    )  # [ds+1, dm]
    col = lambda a: np.ascontiguousarray(f(a).reshape(-1, 1))
    shared = {
        "wxt16": wxt16,
        "wg16": wg16,
        "wob": wob,
        "bd_c": col(bd),
        "nbd_c": col(-f(bd)),
        "bb_c": col(bB),
        "bc_c": col(bC),
        "bx_c": col(bx),
        "ones_row": np.ones((1, Lv), np.float32),
    }
    in_maps = []
    for c in range(NCORES):
        m = dict(shared)
        m["vis16"] = np.ascontiguousarray(vis16[c * BPC : (c + 1) * BPC])
        m["text16"] = np.ascontiguousarray(text16[c * BPC : (c + 1) * BPC])
        in_maps.append(m)
    return in_maps


_PROGRAM_CACHE = {}


def _get_program(mm_dtype=MM_DTYPE):
    key = str(mm_dtype)
    if key not in _PROGRAM_CACHE:
        nc = _build_program(mm_dtype)
        if not nc.is_finalized():
            nc.finalize()
        _PROGRAM_CACHE[key] = nc
    return _PROGRAM_CACHE[key]


def run(inputs: dict, trace: bool = False, mm_dtype=MM_DTYPE):
    """Run the kernel on all 8 cores; returns (full_output, BassKernelResults)."""
    nc = _get_program(mm_dtype)
    in_maps = _prep_host_inputs(**inputs)
    res = run_bass_kernel_spmd(nc, in_maps, list(range(NCORES)), trace=trace)
    full = np.concatenate(
        [res.results[i]["out16"].astype(np.float32) for i in range(NCORES)], axis=0
    )
    return np.ascontiguousarray(full), res


def kernel(**inputs) -> np.ndarray:
    out, _ = run(inputs, trace=False)
    return out
